# revision 9
# baseline (speedup 1.0000x reference)
"""Trainium2 Bass kernel for CompoundMultivariateEmbedding (v4).

Math: out[n] = concat(level_tab[l], type_tab[t], feat_tab[f], exch_tab[e],
pair_tab[p]) @ W.T + b.  Because W is applied to a concat of block lookups,
out[n] = sum_b Ptab_b[idx_b[n]] + b where Ptab_b = tab_b @ W[:, block_b].T.
The bias is folded into the type table (every token hits it exactly once),
the projected tables live in P [92, 128] fp16, and out.T = P.T @ onehot(idx)
is computed on the PE with P stationary.

Per-core pipeline (tokens sharded 8 ways, 131072 tokens/core):
  1. idx int32 -> fp16 via SWDGE cast-DMA into partitions 96-100
  2. SWDGE SBUF->SBUF replication DMAs (0-stride source AP) spread each idx
     row across its vocab-block partitions -> idxrep [92, 8192] fp16
  3. one DVE tensor_scalar is_equal (4x mode: fp16, SBUF, step-1) per
     8192-token half-batch -> multi-hot st [92, 8192] fp16.  Dead vocab rows
     compare against NaN so stale SBUF data can never produce a hit.
  4. per 512 tokens one matmul accumulates P.T @ st into PSUM [128, 512]
  5. PSUM fp32 -> SBUF fp16 evacuation split DVE/ACT (1:2); HWDGE stores
     1 MiB chunks to transposed y [128, n_core] fp16; host untransposes.
"""

import sys

sys.path.insert(0, "/opt/trn_rl_repo")

import numpy as np

import concourse.bass as bass
import concourse.tile as tile
from concourse import bacc, mybir
from concourse._compat import with_exitstack
from contextlib import ExitStack

F32 = mybir.dt.float32
F16 = mybir.dt.float16
I32 = mybir.dt.int32

N_FULL = 1048576
N_CORES = 8
EMBED = 128

TAB_NAMES = ["level_tab", "type_tab", "feature_tab", "exchange_tab", "pair_tab"]
IDX_NAMES = ["level_idx", "type_idx", "feature_idx", "exchange_idx", "pair_idx"]
TAB_ROWS = [50, 2, 2, 3, 20]
TAB_ATTR = [25, 25, 25, 25, 28]
# vocab rows: level 0-49, (dead 50-63), type 64-65 (bias folded in),
# feat 66-67, exch 68-70, pair 71-90, row 91 dead.
VOFF = [0, 64, 66, 68, 71]
FOFF = [0, 25, 50, 75, 100]  # feature (W column) offset per block
V = 92

ST = 1024  # tokens per supertile (one pso tile, 2 PSUM banks)
OSB = 4096  # tokens per output store (1 MiB fp16)
HB = 8192  # tokens per replication half-batch (one big is_equal)
FB = 8192  # tokens per idx DMA batch
DVE_EVAC = 3  # every DVE_EVAC-th supertile evacuates on DVE instead of ACT


@with_exitstack
def _emb_kernel(ctx, tc, y_ap, tabs, w_ap, b_ap, idxs, n_core):
    nc = tc.nc

    const = ctx.enter_context(tc.tile_pool(name="const", bufs=1))

    # ---- helpers for PE transposes ----
    pidx = const.tile([128, 1], I32)
    nc.gpsimd.iota(pidx, pattern=[[0, 1]], base=0, channel_multiplier=1)
    pidx_f = const.tile([128, 1], F32)
    nc.vector.tensor_copy(pidx_f, pidx)
    iotaf = const.tile([128, 128], I32)
    nc.gpsimd.iota(iotaf, pattern=[[1, 128]], base=0, channel_multiplier=0)
    ident = const.tile([128, 128], F32)
    nc.vector.tensor_scalar(ident, iotaf, pidx_f[:, :], None, mybir.AluOpType.is_equal)

    # ---- setup: projected tables (PSUM pool closed before the main loop) ----
    setup = ExitStack()
    psum_set = setup.enter_context(
        tc.tile_pool(name="psum_set", bufs=1, space=bass.MemorySpace.PSUM)
    )

    # W^T
    w_sb = const.tile([128, 128], F32)
    nc.sync.dma_start(w_sb, w_ap)
    psum_wt = psum_set.tile([128, 128], F32, tag="pset")
    nc.tensor.transpose(psum_wt, w_sb, ident)
    wt_sb = const.tile([128, 128], F32)
    nc.scalar.copy(wt_sb, psum_wt)

    # projected tables -> pf32 [92, 128]
    pf32 = const.tile([V, EMBED], F32)
    nc.vector.memset(pf32, 0.0)
    for j in range(5):
        rows, attr = TAB_ROWS[j], TAB_ATTR[j]
        tab_sb = const.tile([rows, attr], F32, name=f"tab{j}")
        nc.sync.dma_start(tab_sb, tabs[j])
        psum_tt = psum_set.tile([attr, rows], F32, tag="pset", name=f"ptt{j}")
        nc.tensor.transpose(psum_tt, tab_sb, ident[0:rows, 0:rows])
        tabt_sb = const.tile([attr, rows], F32, name=f"tabt{j}")
        nc.scalar.copy(tabt_sb, psum_tt)
        wb_sb = const.tile([attr, EMBED], F32, name=f"wb{j}")
        nc.gpsimd.dma_start(wb_sb, wt_sb[FOFF[j] : FOFF[j] + attr, :])
        psum_pb = psum_set.tile([rows, EMBED], F32, tag="pset", name=f"ppb{j}")
        nc.tensor.matmul(psum_pb, tabt_sb, wb_sb)
        pb_sb = const.tile([rows, EMBED], F32, name=f"pb{j}")
        nc.scalar.copy(pb_sb, psum_pb)
        nc.gpsimd.dma_start(pf32[VOFF[j] : VOFF[j] + rows, :], pb_sb)
    # fold bias into the two type-table rows (each token hits exactly one)
    bb2 = const.tile([66, EMBED], F32)
    for r in range(64, 66):
        nc.sync.dma_start(bb2[r : r + 1, :], b_ap)
    nc.vector.tensor_add(pf32[64:66, :], pf32[64:66, :], bb2[64:66, :])

    pf16 = const.tile([V, EMBED], F16)
    nc.vector.tensor_copy(pf16, pf32)

    setup.close()  # free setup PSUM banks

    # ---- iota column (fp16): within-block index per vocab partition ----
    # dead rows get NaN so is_equal can never hit on stale idxrep data
    off_row = const.tile([1, 128], F32)
    nc.vector.memset(off_row, float("nan"))
    for j in range(5):
        nc.vector.memset(off_row[:, VOFF[j] : VOFF[j] + TAB_ROWS[j]], float(VOFF[j]))
    offc = const.tile([128, 1], F32)
    nc.gpsimd.dma_start(offc, off_row)
    iota_f32 = const.tile([128, 1], F32)
    nc.vector.tensor_sub(iota_f32, pidx_f, offc)

    # ---- main loop ----
    idx_pool = ctx.enter_context(tc.tile_pool(name="idxp", bufs=2))
    rep_pool = ctx.enter_context(tc.tile_pool(name="repp", bufs=2))
    strep_pool = ctx.enter_context(tc.tile_pool(name="strp", bufs=2))
    out_pool = ctx.enter_context(tc.tile_pool(name="outp", bufs=2))
    pso_pool = ctx.enter_context(
        tc.tile_pool(name="pout", bufs=4, space=bass.MemorySpace.PSUM)
    )

    n_sup = n_core // ST
    sup_per_hb = HB // ST
    sup_per_osb = OSB // ST
    n_half = n_core // HB
    idxf_tiles = {}

    def load_idx_batch(b):
        t = idx_pool.tile([128, FB], F16, name=f"idxf{b % 2}")
        for j in range(5):
            # SWDGE cast-DMA: int32 -> fp16 in flight
            nc.gpsimd.dma_start(t[96 + j : 97 + j, :], idxs[j][b * FB : (b + 1) * FB])
        idxf_tiles[b] = t

    # doubling-ladder replication: every row within a block is an identical
    # copy of that block's idx stream, so range doublings replicate it.
    # (dst_row, src_row, nrows); seeds use src_row >= 96 (the idxf tile).
    LADDER = [
        (0, 96, 1), (1, 0, 1), (2, 0, 2), (4, 0, 4), (8, 0, 8),
        (16, 0, 16), (32, 0, 18),                      # level -> 0-49
        (64, 97, 1), (65, 64, 1),                      # type -> 64-65
        (66, 98, 1), (67, 66, 1),                      # feat -> 66-67
        (68, 99, 1), (69, 68, 1), (70, 68, 1),         # exch -> 68-70
        (71, 100, 1), (72, 71, 1), (73, 71, 2), (75, 71, 4),
        (79, 71, 8), (87, 71, 4),                      # pair -> 71-90
    ]

    def prep_half(h):
        """Replicate idx rows across vocab partitions, then one 4x is_equal."""
        b = (h * HB) // FB
        if b not in idxf_tiles:
            load_idx_batch(b)
        off = (h * HB) % FB
        rep = rep_pool.tile([V, HB], F16)
        for dst, src, n in LADDER:
            if src >= 96:
                s = idxf_tiles[b][src : src + n, off : off + HB]
            else:
                s = rep[src : src + n, :]
            nc.sync.dma_start(rep[dst : dst + n, :], s)
        strep = strep_pool.tile([V, HB], F16)
        nc.vector.tensor_scalar(
            strep, rep, iota_f32[0:V, :], None, mybir.AluOpType.is_equal
        )
        return strep

    strep_cur = prep_half(0)
    strep_nxt = None
    osb = None
    for k in range(n_sup):
        h, s = divmod(k, sup_per_hb)
        if s == 3 and h + 1 < n_half:
            strep_nxt = prep_half(h + 1)
        pso = pso_pool.tile([128, ST], F32)
        for c in range(ST // 512):
            nc.tensor.matmul(
                pso[:, bass.ts(c, 512)],
                pf16,
                strep_cur[:, s * ST + c * 512 : s * ST + (c + 1) * 512],
            )
        if k % sup_per_osb == 0:
            osb = out_pool.tile([128, OSB], F16)
        dst = osb[:, (k % sup_per_osb) * ST : (k % sup_per_osb + 1) * ST]
        if k % DVE_EVAC == DVE_EVAC - 1:
            nc.vector.tensor_copy(dst, pso)
        else:
            nc.scalar.copy(dst, pso)
        if (k + 1) % sup_per_osb == 0:
            n0 = (k + 1 - sup_per_osb) * ST
            nc.sync.dma_start(y_ap[:, n0 : n0 + OSB], osb)
        if s == sup_per_hb - 1:
            strep_cur = strep_nxt


def build(n_core, num_devices=N_CORES):
    nc = bacc.Bacc(
        "TRN2", target_bir_lowering=False, debug=False, num_devices=num_devices
    )
    tabs, idxs = [], []
    for j, nm in enumerate(TAB_NAMES):
        tabs.append(nc.dram_tensor(nm, [TAB_ROWS[j], TAB_ATTR[j]], F32,
                                   kind="ExternalInput").ap())
    w_ap = nc.dram_tensor("W", [EMBED, EMBED], F32, kind="ExternalInput").ap()
    b_ap = nc.dram_tensor("b", [EMBED], F32, kind="ExternalInput").ap()
    for nm in IDX_NAMES:
        idxs.append(nc.dram_tensor(nm, [n_core], I32, kind="ExternalInput").ap())
    # transposed output: [embed, tokens] fp16; host transposes + upcasts
    y = nc.dram_tensor("y", [EMBED, n_core], F16, kind="ExternalOutput")

    with tile.TileContext(nc) as tc:
        _emb_kernel(tc, y.ap(), tabs, w_ap, b_ap, idxs, n_core)
    nc.compile()
    return nc


_NC_CACHE = {}


def _get_nc(n_core):
    if n_core not in _NC_CACHE:
        _NC_CACHE[n_core] = build(n_core)
    return _NC_CACHE[n_core]


def _make_in_maps(inputs, n_cores, n_core):
    shared = {}
    for nm in TAB_NAMES + ["W", "b"]:
        shared[nm] = np.ascontiguousarray(np.asarray(inputs[nm], dtype=np.float32))
    in_maps = []
    for c in range(n_cores):
        m = dict(shared)
        for nm in IDX_NAMES:
            m[nm] = np.ascontiguousarray(
                np.asarray(inputs[nm], dtype=np.int32)[c * n_core : (c + 1) * n_core]
            )
        in_maps.append(m)
    return in_maps


TRACE_DIR = "/tmp/bass_trace"


def run(inputs, trace=False, tmpdir=None):
    """Run on hardware across 8 cores; returns (full_output, BassKernelResults)."""
    from concourse.bass_utils import run_bass_kernel_spmd

    n = np.asarray(inputs[IDX_NAMES[0]]).shape[0]
    n_core = n // N_CORES
    nc = _get_nc(n_core)
    in_maps = _make_in_maps(inputs, N_CORES, n_core)
    if trace and tmpdir is not None:
        import os
        import shutil

        shutil.rmtree(tmpdir, ignore_errors=True)
        os.makedirs(tmpdir, exist_ok=True)
    res = run_bass_kernel_spmd(nc, in_maps, core_ids=list(range(N_CORES)),
                               trace=trace, tmpdir=tmpdir if trace else None)
    out = np.empty((n, EMBED), dtype=np.float32)
    for c in range(N_CORES):
        out[c * n_core : (c + 1) * n_core, :] = res.results[c]["y"].T
    return out, res


def kernel(**inputs):
    out, _ = run(inputs)
    return out


# revision 11
# speedup vs baseline: 2.0273x; 2.0273x over previous
"""Trainium2 Bass kernel for CompoundMultivariateEmbedding (v6).

Math: out[n] = concat(level_tab[l], type_tab[t], feat_tab[f], exch_tab[e],
pair_tab[p]) @ W.T + b.  Because W is applied to a concat of block lookups,
out[n] = sum_b Ptab_b[idx_b[n]] + b where Ptab_b = tab_b @ W[:, block_b].T.

Vocab layout (K=75): level one-hot rows 0-49, pair one-hot rows 50-69,
exchange one-hot rows 70-72, plus two DIRECT rows: the 2-row type/feature
tables are affine in their index (P[t] = P[0] + t*delta), so row 73 carries
raw type_idx against delta_type and row 74 raw feat_idx against delta_feat.
The constant terms (bias + P_type[0] + P_feat[0]) are folded into the three
exchange rows (every token hits exactly one).  out.T = P.T @ st on the PE
with P [75, 128] fp16 stationary.

The host uploads idx16 [29, n_core] fp16: level idx x16, pair x8, exch x3
seed copies plus type/feat (a pure input-layout transform).  Per 8192-token
half-batch the device issues 4 strided loads + 5 depth-1 doubling DMAs to
fan the seeds out to all one-hot rows, then ONE DVE tensor_scalar is_equal
(4x mode: fp16 SBUF step-1) builds the multi-hot st [75, 8192].

Per 512 tokens one matmul accumulates P.T @ st into PSUM [128, 512]; the
PSUM fp32 -> SBUF fp16 evacuation is split DVE/ACT (1:2); HWDGE stores
1 MiB chunks to transposed y [128, n_core] fp16; the host untransposes.
"""

import sys

sys.path.insert(0, "/opt/trn_rl_repo")

import numpy as np

import concourse.bass as bass
import concourse.tile as tile
from concourse import bacc, mybir
from concourse._compat import with_exitstack
from contextlib import ExitStack

F32 = mybir.dt.float32
F16 = mybir.dt.float16

N_FULL = 1048576
N_CORES = 8
EMBED = 128

TAB_NAMES = ["level_tab", "type_tab", "feature_tab", "exchange_tab", "pair_tab"]
IDX_NAMES = ["level_idx", "type_idx", "feature_idx", "exchange_idx", "pair_idx"]
TAB_ROWS = [50, 2, 2, 3, 20]
TAB_ATTR = [25, 25, 25, 25, 28]
FOFF = [0, 25, 50, 75, 100]  # feature (W column) offset per block
# one-hot vocab offsets for level / exchange / pair
LVL0, PAIR0, EXCH0 = 0, 50, 70
DT_ROW, DF_ROW = 73, 74  # direct rows: raw type_idx, feat_idx
V = 75

# host-side idx16 seed layout: level x16, pair x8, exch x3, type, feat
SEED_LVL, SEED_PAIR, SEED_EXCH = 16, 8, 3
IDX16_ROWS = SEED_LVL + SEED_PAIR + SEED_EXCH + 2  # 29

ST = 1024  # tokens per supertile (one pso tile, 2 PSUM banks)
OSB = 4096  # tokens per output store (1 MiB fp16)
HB = 8192  # tokens per half-batch (one big is_equal)
DVE_EVAC = 3  # every DVE_EVAC-th supertile evacuates on DVE instead of ACT


@with_exitstack
def _emb_kernel(ctx, tc, y_ap, tabs, w_ap, b_ap, idx16, n_core):
    nc = tc.nc

    const = ctx.enter_context(tc.tile_pool(name="const", bufs=1))

    # ---- helpers for PE transposes ----
    pidx = const.tile([128, 1], mybir.dt.int32)
    nc.gpsimd.iota(pidx, pattern=[[0, 1]], base=0, channel_multiplier=1)
    pidx_f = const.tile([128, 1], F32)
    nc.vector.tensor_copy(pidx_f, pidx)
    iotaf = const.tile([128, 128], mybir.dt.int32)
    nc.gpsimd.iota(iotaf, pattern=[[1, 128]], base=0, channel_multiplier=0)
    ident = const.tile([128, 128], F32)
    nc.vector.tensor_scalar(ident, iotaf, pidx_f[:, :], None, mybir.AluOpType.is_equal)

    # ---- setup: projected tables (PSUM pool closed before the main loop) ----
    setup = ExitStack()
    psum_set = setup.enter_context(
        tc.tile_pool(name="psum_set", bufs=1, space=bass.MemorySpace.PSUM)
    )

    # W^T
    w_sb = const.tile([128, 128], F32)
    nc.sync.dma_start(w_sb, w_ap)
    psum_wt = psum_set.tile([128, 128], F32, tag="pset")
    nc.tensor.transpose(psum_wt, w_sb, ident)
    wt_sb = const.tile([128, 128], F32)
    nc.scalar.copy(wt_sb, psum_wt)

    # projected tables -> pf32 [75, 128]
    pf32 = const.tile([V, EMBED], F32)
    nc.vector.memset(pf32, 0.0)
    first_rows = {}  # j -> [1,128] tile holding P_j[0] (for type/feat fold)
    onehot_off = {0: LVL0, 3: EXCH0, 4: PAIR0}
    for j in range(5):
        rows, attr = TAB_ROWS[j], TAB_ATTR[j]
        tab_sb = const.tile([rows, attr], F32, name=f"tab{j}")
        nc.sync.dma_start(tab_sb, tabs[j])
        psum_tt = psum_set.tile([attr, rows], F32, tag="pset", name=f"ptt{j}")
        nc.tensor.transpose(psum_tt, tab_sb, ident[0:rows, 0:rows])
        tabt_sb = const.tile([attr, rows], F32, name=f"tabt{j}")
        nc.scalar.copy(tabt_sb, psum_tt)
        wb_sb = const.tile([attr, EMBED], F32, name=f"wb{j}")
        nc.gpsimd.dma_start(wb_sb, wt_sb[FOFF[j] : FOFF[j] + attr, :])
        psum_pb = psum_set.tile([rows, EMBED], F32, tag="pset", name=f"ppb{j}")
        nc.tensor.matmul(psum_pb, tabt_sb, wb_sb)
        pb_sb = const.tile([rows, EMBED], F32, name=f"pb{j}")
        nc.scalar.copy(pb_sb, psum_pb)
        if j in (1, 2):  # type / feature: direct affine rows
            rA = const.tile([1, EMBED], F32, name=f"rA{j}")
            nc.gpsimd.dma_start(rA, pb_sb[0:1, :])
            rB = const.tile([1, EMBED], F32, name=f"rB{j}")
            nc.gpsimd.dma_start(rB, pb_sb[1:2, :])
            delta = const.tile([1, EMBED], F32, name=f"dl{j}")
            nc.vector.tensor_sub(delta, rB, rA)
            row = DT_ROW if j == 1 else DF_ROW
            nc.gpsimd.dma_start(pf32[row : row + 1, :], delta)
            first_rows[j] = rA
        elif j == 3:
            # fold bias + P_type[0] + P_feat[0] into the three exchange
            # rows while they still sit at partitions 0-2 (32-aligned)
            b_row = const.tile([1, EMBED], F32)
            nc.sync.dma_start(b_row, b_ap)
            comb = const.tile([1, EMBED], F32)
            nc.vector.tensor_add(comb, first_rows[1], first_rows[2])
            nc.vector.tensor_add(comb, comb, b_row)
            comb3 = const.tile([3, EMBED], F32)
            for r in range(3):
                nc.gpsimd.dma_start(comb3[r : r + 1, :], comb)
            nc.vector.tensor_add(pb_sb, pb_sb, comb3)
            nc.gpsimd.dma_start(pf32[EXCH0 : EXCH0 + 3, :], pb_sb)
        else:
            off = onehot_off[j]
            nc.gpsimd.dma_start(pf32[off : off + rows, :], pb_sb)

    pf16 = const.tile([V, EMBED], F16)
    nc.vector.tensor_copy(pf16, pf32)

    setup.close()  # free setup PSUM banks

    # ---- iota column: within-block index per one-hot vocab partition ----
    off_row = const.tile([1, 128], F32)
    nc.vector.memset(off_row, float("nan"))
    nc.vector.memset(off_row[:, LVL0 : LVL0 + 50], float(LVL0))
    nc.vector.memset(off_row[:, PAIR0 : PAIR0 + 20], float(PAIR0))
    nc.vector.memset(off_row[:, EXCH0 : EXCH0 + 3], float(EXCH0))
    offc = const.tile([128, 1], F32)
    nc.gpsimd.dma_start(offc, off_row)
    iota_f32 = const.tile([128, 1], F32)
    nc.vector.tensor_sub(iota_f32, pidx_f, offc)

    # ---- main loop ----
    rep_pool = ctx.enter_context(tc.tile_pool(name="repp", bufs=2))
    st_pool = ctx.enter_context(tc.tile_pool(name="stp", bufs=2))
    out_pool = ctx.enter_context(tc.tile_pool(name="outp", bufs=2))
    pso_pool = ctx.enter_context(
        tc.tile_pool(name="pout", bufs=4, space=bass.MemorySpace.PSUM)
    )

    n_sup = n_core // ST
    sup_per_hb = HB // ST
    sup_per_osb = OSB // ST
    n_half = n_core // HB

    # seed fan-out: (dst, src, nrows) range copies within the rep tile;
    # every row within a block is an identical copy so any range works.
    LADDER = [
        (16, 0, 16), (32, 0, 16), (48, 0, 2),   # level 16 seeds -> rows 0-49
        (58, 50, 8), (66, 50, 4),               # pair 8 seeds -> rows 50-69
    ]

    def prep_half(h):
        off = h * HB
        rep = rep_pool.tile([73, HB], F16)
        st = st_pool.tile([V, HB], F16)
        nc.sync.dma_start(rep[0:16, :], idx16[0:16, off : off + HB])
        nc.sync.dma_start(rep[50:58, :], idx16[16:24, off : off + HB])
        nc.sync.dma_start(rep[70:73, :], idx16[24:27, off : off + HB])
        nc.sync.dma_start(st[DT_ROW : DT_ROW + 2, :], idx16[27:29, off : off + HB])
        for dst, src, n in LADDER:
            nc.scalar.dma_start(rep[dst : dst + n, :], rep[src : src + n, :])
        nc.vector.tensor_scalar(
            st[0:73, :], rep, iota_f32[0:73, :], None, mybir.AluOpType.is_equal
        )
        return st

    st_cur = prep_half(0)
    st_nxt = None
    osb = None
    for k in range(n_sup):
        h, s = divmod(k, sup_per_hb)
        if s == 3 and h + 1 < n_half:
            st_nxt = prep_half(h + 1)
        pso = pso_pool.tile([128, ST], F32)
        for c in range(ST // 512):
            nc.tensor.matmul(
                pso[:, bass.ts(c, 512)],
                pf16,
                st_cur[:, s * ST + c * 512 : s * ST + (c + 1) * 512],
            )
        if k % sup_per_osb == 0:
            osb = out_pool.tile([128, OSB], F16)
        dst = osb[:, (k % sup_per_osb) * ST : (k % sup_per_osb + 1) * ST]
        if k % DVE_EVAC == DVE_EVAC - 1:
            nc.vector.tensor_copy(dst, pso)
        else:
            nc.scalar.copy(dst, pso)
        if (k + 1) % sup_per_osb == 0:
            n0 = (k + 1 - sup_per_osb) * ST
            eng = nc.sync if (k // sup_per_osb) % 2 == 0 else nc.scalar
            eng.dma_start(y_ap[:, n0 : n0 + OSB], osb)
        if s == sup_per_hb - 1:
            st_cur = st_nxt


def build(n_core, num_devices=N_CORES):
    nc = bacc.Bacc(
        "TRN2", target_bir_lowering=False, debug=False, num_devices=num_devices
    )
    tabs = []
    for j, nm in enumerate(TAB_NAMES):
        tabs.append(nc.dram_tensor(nm, [TAB_ROWS[j], TAB_ATTR[j]], F32,
                                   kind="ExternalInput").ap())
    w_ap = nc.dram_tensor("W", [EMBED, EMBED], F32, kind="ExternalInput").ap()
    b_ap = nc.dram_tensor("b", [EMBED], F32, kind="ExternalInput").ap()
    idx16 = nc.dram_tensor("idx16", [IDX16_ROWS, n_core], F16,
                           kind="ExternalInput").ap()
    # transposed output: [embed, tokens] fp16; host transposes + upcasts
    y = nc.dram_tensor("y", [EMBED, n_core], F16, kind="ExternalOutput")

    with tile.TileContext(nc) as tc:
        _emb_kernel(tc, y.ap(), tabs, w_ap, b_ap, idx16, n_core)
    nc.compile()
    return nc


_NC_CACHE = {}


def _get_nc(n_core):
    if n_core not in _NC_CACHE:
        _NC_CACHE[n_core] = build(n_core)
    return _NC_CACHE[n_core]


def _make_in_maps(inputs, n_cores, n_core):
    shared = {}
    for nm in TAB_NAMES + ["W", "b"]:
        shared[nm] = np.ascontiguousarray(np.asarray(inputs[nm], dtype=np.float32))
    idx = {nm: np.asarray(inputs[nm], dtype=np.int32) for nm in IDX_NAMES}
    in_maps = []
    for c in range(n_cores):
        m = dict(shared)
        sl = slice(c * n_core, (c + 1) * n_core)
        a = np.empty((IDX16_ROWS, n_core), np.float16)
        a[0:16] = idx["level_idx"][sl].astype(np.float16)[None, :]
        a[16:24] = idx["pair_idx"][sl].astype(np.float16)[None, :]
        a[24:27] = idx["exchange_idx"][sl].astype(np.float16)[None, :]
        a[27] = idx["type_idx"][sl].astype(np.float16)
        a[28] = idx["feature_idx"][sl].astype(np.float16)
        m["idx16"] = a
        in_maps.append(m)
    return in_maps


TRACE_DIR = "/tmp/bass_trace"


def run(inputs, trace=False, tmpdir=None):
    """Run on hardware across 8 cores; returns (full_output, BassKernelResults)."""
    from concourse.bass_utils import run_bass_kernel_spmd

    n = np.asarray(inputs[IDX_NAMES[0]]).shape[0]
    n_core = n // N_CORES
    nc = _get_nc(n_core)
    in_maps = _make_in_maps(inputs, N_CORES, n_core)
    if trace and tmpdir is not None:
        import os
        import shutil

        shutil.rmtree(tmpdir, ignore_errors=True)
        os.makedirs(tmpdir, exist_ok=True)
    res = run_bass_kernel_spmd(nc, in_maps, core_ids=list(range(N_CORES)),
                               trace=trace, tmpdir=tmpdir if trace else None)
    out = np.empty((n, EMBED), dtype=np.float32)
    for c in range(N_CORES):
        out[c * n_core : (c + 1) * n_core, :] = res.results[c]["y"].T
    return out, res


def kernel(**inputs):
    out, _ = run(inputs)
    return out


# revision 12
# speedup vs baseline: 2.0487x; 1.0106x over previous
"""Trainium2 Bass kernel for CompoundMultivariateEmbedding (v6).

Math: out[n] = concat(level_tab[l], type_tab[t], feat_tab[f], exch_tab[e],
pair_tab[p]) @ W.T + b.  Because W is applied to a concat of block lookups,
out[n] = sum_b Ptab_b[idx_b[n]] + b where Ptab_b = tab_b @ W[:, block_b].T.

Vocab layout (K=75): level one-hot rows 0-49, pair one-hot rows 50-69,
exchange one-hot rows 70-72, plus two DIRECT rows: the 2-row type/feature
tables are affine in their index (P[t] = P[0] + t*delta), so row 73 carries
raw type_idx against delta_type and row 74 raw feat_idx against delta_feat.
The constant terms (bias + P_type[0] + P_feat[0]) are folded into the three
exchange rows (every token hits exactly one).  out.T = P.T @ st on the PE
with P [75, 128] fp16 stationary.

The host uploads idx16 [29, n_core] fp16: level idx x16, pair x8, exch x3
seed copies plus type/feat (a pure input-layout transform).  Per 8192-token
half-batch the device issues 4 strided loads + 5 depth-1 doubling DMAs to
fan the seeds out to all one-hot rows, then ONE DVE tensor_scalar is_equal
(4x mode: fp16 SBUF step-1) builds the multi-hot st [75, 8192].

Per 512 tokens one matmul accumulates P.T @ st into PSUM [128, 512]; the
PSUM fp32 -> SBUF fp16 evacuation is split DVE/ACT (1:2); HWDGE stores
1 MiB chunks to transposed y [128, n_core] fp16; the host untransposes.
"""

import sys

sys.path.insert(0, "/opt/trn_rl_repo")

import numpy as np

import concourse.bass as bass
import concourse.tile as tile
from concourse import bacc, mybir
from concourse._compat import with_exitstack
from contextlib import ExitStack

F32 = mybir.dt.float32
F16 = mybir.dt.float16

N_FULL = 1048576
N_CORES = 8
EMBED = 128

TAB_NAMES = ["level_tab", "type_tab", "feature_tab", "exchange_tab", "pair_tab"]
IDX_NAMES = ["level_idx", "type_idx", "feature_idx", "exchange_idx", "pair_idx"]
TAB_ROWS = [50, 2, 2, 3, 20]
TAB_ATTR = [25, 25, 25, 25, 28]
FOFF = [0, 25, 50, 75, 100]  # feature (W column) offset per block
# one-hot vocab offsets for level / exchange / pair
LVL0, PAIR0, EXCH0 = 0, 50, 70
DT_ROW, DF_ROW = 73, 74  # direct rows: raw type_idx, feat_idx
V = 75

# host-side idx16 seed layout: level x16, pair x8, exch x3, type, feat
SEED_LVL, SEED_PAIR, SEED_EXCH = 16, 8, 3
IDX16_ROWS = SEED_LVL + SEED_PAIR + SEED_EXCH + 2  # 29

ST = 1024  # tokens per supertile (one pso tile, 2 PSUM banks)
OSB = 4096  # tokens per output store (1 MiB fp16)
HB = 8192  # tokens per half-batch (one big is_equal)
DVE_EVAC = 3  # every DVE_EVAC-th supertile evacuates on DVE instead of ACT


@with_exitstack
def _emb_kernel(ctx, tc, y_ap, tabs, w_ap, b_ap, idx16, n_core):
    nc = tc.nc

    const = ctx.enter_context(tc.tile_pool(name="const", bufs=1))

    # ---- helpers for PE transposes ----
    pidx = const.tile([128, 1], mybir.dt.int32)
    nc.gpsimd.iota(pidx, pattern=[[0, 1]], base=0, channel_multiplier=1)
    pidx_f = const.tile([128, 1], F32)
    nc.vector.tensor_copy(pidx_f, pidx)
    iotaf = const.tile([128, 128], mybir.dt.int32)
    nc.gpsimd.iota(iotaf, pattern=[[1, 128]], base=0, channel_multiplier=0)
    ident = const.tile([128, 128], F32)
    nc.vector.tensor_scalar(ident, iotaf, pidx_f[:, :], None, mybir.AluOpType.is_equal)

    # ---- setup: projected tables (PSUM pool closed before the main loop) ----
    setup = ExitStack()
    psum_set = setup.enter_context(
        tc.tile_pool(name="psum_set", bufs=1, space=bass.MemorySpace.PSUM)
    )

    # W^T
    w_sb = const.tile([128, 128], F32)
    nc.sync.dma_start(w_sb, w_ap)
    psum_wt = psum_set.tile([128, 128], F32, tag="pset")
    nc.tensor.transpose(psum_wt, w_sb, ident)
    wt_sb = const.tile([128, 128], F32)
    nc.scalar.copy(wt_sb, psum_wt)

    # projected tables -> pf32 [75, 128]
    pf32 = const.tile([V, EMBED], F32)
    nc.vector.memset(pf32, 0.0)
    first_rows = {}  # j -> [1,128] tile holding P_j[0] (for type/feat fold)
    onehot_off = {0: LVL0, 3: EXCH0, 4: PAIR0}
    for j in range(5):
        rows, attr = TAB_ROWS[j], TAB_ATTR[j]
        tab_sb = const.tile([rows, attr], F32, name=f"tab{j}")
        nc.sync.dma_start(tab_sb, tabs[j])
        psum_tt = psum_set.tile([attr, rows], F32, tag="pset", name=f"ptt{j}")
        nc.tensor.transpose(psum_tt, tab_sb, ident[0:rows, 0:rows])
        tabt_sb = const.tile([attr, rows], F32, name=f"tabt{j}")
        nc.scalar.copy(tabt_sb, psum_tt)
        wb_sb = const.tile([attr, EMBED], F32, name=f"wb{j}")
        nc.gpsimd.dma_start(wb_sb, wt_sb[FOFF[j] : FOFF[j] + attr, :])
        psum_pb = psum_set.tile([rows, EMBED], F32, tag="pset", name=f"ppb{j}")
        nc.tensor.matmul(psum_pb, tabt_sb, wb_sb)
        pb_sb = const.tile([rows, EMBED], F32, name=f"pb{j}")
        nc.scalar.copy(pb_sb, psum_pb)
        if j in (1, 2):  # type / feature: direct affine rows
            rA = const.tile([1, EMBED], F32, name=f"rA{j}")
            nc.gpsimd.dma_start(rA, pb_sb[0:1, :])
            rB = const.tile([1, EMBED], F32, name=f"rB{j}")
            nc.gpsimd.dma_start(rB, pb_sb[1:2, :])
            delta = const.tile([1, EMBED], F32, name=f"dl{j}")
            nc.vector.tensor_sub(delta, rB, rA)
            row = DT_ROW if j == 1 else DF_ROW
            nc.gpsimd.dma_start(pf32[row : row + 1, :], delta)
            first_rows[j] = rA
        elif j == 3:
            # fold bias + P_type[0] + P_feat[0] into the three exchange
            # rows while they still sit at partitions 0-2 (32-aligned)
            b_row = const.tile([1, EMBED], F32)
            nc.sync.dma_start(b_row, b_ap)
            comb = const.tile([1, EMBED], F32)
            nc.vector.tensor_add(comb, first_rows[1], first_rows[2])
            nc.vector.tensor_add(comb, comb, b_row)
            comb3 = const.tile([3, EMBED], F32)
            for r in range(3):
                nc.gpsimd.dma_start(comb3[r : r + 1, :], comb)
            nc.vector.tensor_add(pb_sb, pb_sb, comb3)
            nc.gpsimd.dma_start(pf32[EXCH0 : EXCH0 + 3, :], pb_sb)
        else:
            off = onehot_off[j]
            nc.gpsimd.dma_start(pf32[off : off + rows, :], pb_sb)

    pf16 = const.tile([V, EMBED], F16)
    nc.vector.tensor_copy(pf16, pf32)

    setup.close()  # free setup PSUM banks

    # ---- iota column: within-block index per one-hot vocab partition ----
    off_row = const.tile([1, 128], F32)
    nc.vector.memset(off_row, float("nan"))
    nc.vector.memset(off_row[:, LVL0 : LVL0 + 50], float(LVL0))
    nc.vector.memset(off_row[:, PAIR0 : PAIR0 + 20], float(PAIR0))
    nc.vector.memset(off_row[:, EXCH0 : EXCH0 + 3], float(EXCH0))
    offc = const.tile([128, 1], F32)
    nc.gpsimd.dma_start(offc, off_row)
    iota_f32 = const.tile([128, 1], F32)
    nc.vector.tensor_sub(iota_f32, pidx_f, offc)

    # ---- main loop ----
    rep_pool = ctx.enter_context(tc.tile_pool(name="repp", bufs=3))
    st_pool = ctx.enter_context(tc.tile_pool(name="stp", bufs=3))
    out_pool = ctx.enter_context(tc.tile_pool(name="outp", bufs=2))
    pso_pool = ctx.enter_context(
        tc.tile_pool(name="pout", bufs=4, space=bass.MemorySpace.PSUM)
    )

    n_sup = n_core // ST
    sup_per_hb = HB // ST
    sup_per_osb = OSB // ST
    n_half = n_core // HB

    # seed fan-out: (dst, src, nrows) range copies within the rep tile;
    # every row within a block is an identical copy so any range works.
    LADDER = [
        (16, 0, 16), (32, 0, 16), (48, 0, 2),   # level 16 seeds -> rows 0-49
        (58, 50, 8), (66, 50, 4),               # pair 8 seeds -> rows 50-69
    ]

    def prep_half(h):
        off = h * HB
        rep = rep_pool.tile([73, HB], F16)
        st = st_pool.tile([V, HB], F16)
        nc.scalar.dma_start(rep[0:16, :], idx16[0:16, off : off + HB])
        nc.scalar.dma_start(rep[50:58, :], idx16[16:24, off : off + HB])
        nc.scalar.dma_start(rep[70:73, :], idx16[24:27, off : off + HB])
        nc.scalar.dma_start(st[DT_ROW : DT_ROW + 2, :], idx16[27:29, off : off + HB])
        for dst, src, n in LADDER:
            nc.scalar.dma_start(rep[dst : dst + n, :], rep[src : src + n, :])
        nc.vector.tensor_scalar(
            st[0:73, :], rep, iota_f32[0:73, :], None, mybir.AluOpType.is_equal
        )
        return st

    st_tiles = {0: prep_half(0)}
    if n_half > 1:
        st_tiles[1] = prep_half(1)
    osb = None
    for k in range(n_sup):
        h, s = divmod(k, sup_per_hb)
        if s == 0 and h + 2 < n_half:
            st_tiles[h + 2] = prep_half(h + 2)
        st_cur = st_tiles[h]
        pso = pso_pool.tile([128, ST], F32)
        for c in range(ST // 512):
            nc.tensor.matmul(
                pso[:, bass.ts(c, 512)],
                pf16,
                st_cur[:, s * ST + c * 512 : s * ST + (c + 1) * 512],
            )
        if k % sup_per_osb == 0:
            osb = out_pool.tile([128, OSB], F16)
        dst = osb[:, (k % sup_per_osb) * ST : (k % sup_per_osb + 1) * ST]
        if k % DVE_EVAC == DVE_EVAC - 1:
            nc.vector.tensor_copy(dst, pso)
        else:
            nc.scalar.copy(dst, pso)
        if (k + 1) % sup_per_osb == 0:
            n0 = (k + 1 - sup_per_osb) * ST
            nc.sync.dma_start(y_ap[:, n0 : n0 + OSB], osb)
        if s == sup_per_hb - 1:
            st_tiles.pop(h, None)


def build(n_core, num_devices=N_CORES):
    nc = bacc.Bacc(
        "TRN2", target_bir_lowering=False, debug=False, num_devices=num_devices
    )
    tabs = []
    for j, nm in enumerate(TAB_NAMES):
        tabs.append(nc.dram_tensor(nm, [TAB_ROWS[j], TAB_ATTR[j]], F32,
                                   kind="ExternalInput").ap())
    w_ap = nc.dram_tensor("W", [EMBED, EMBED], F32, kind="ExternalInput").ap()
    b_ap = nc.dram_tensor("b", [EMBED], F32, kind="ExternalInput").ap()
    idx16 = nc.dram_tensor("idx16", [IDX16_ROWS, n_core], F16,
                           kind="ExternalInput").ap()
    # transposed output: [embed, tokens] fp16; host transposes + upcasts
    y = nc.dram_tensor("y", [EMBED, n_core], F16, kind="ExternalOutput")

    with tile.TileContext(nc) as tc:
        _emb_kernel(tc, y.ap(), tabs, w_ap, b_ap, idx16, n_core)
    nc.compile()
    return nc


_NC_CACHE = {}


def _get_nc(n_core):
    if n_core not in _NC_CACHE:
        _NC_CACHE[n_core] = build(n_core)
    return _NC_CACHE[n_core]


def _make_in_maps(inputs, n_cores, n_core):
    shared = {}
    for nm in TAB_NAMES + ["W", "b"]:
        shared[nm] = np.ascontiguousarray(np.asarray(inputs[nm], dtype=np.float32))
    idx = {nm: np.asarray(inputs[nm], dtype=np.int32) for nm in IDX_NAMES}
    in_maps = []
    for c in range(n_cores):
        m = dict(shared)
        sl = slice(c * n_core, (c + 1) * n_core)
        a = np.empty((IDX16_ROWS, n_core), np.float16)
        a[0:16] = idx["level_idx"][sl].astype(np.float16)[None, :]
        a[16:24] = idx["pair_idx"][sl].astype(np.float16)[None, :]
        a[24:27] = idx["exchange_idx"][sl].astype(np.float16)[None, :]
        a[27] = idx["type_idx"][sl].astype(np.float16)
        a[28] = idx["feature_idx"][sl].astype(np.float16)
        m["idx16"] = a
        in_maps.append(m)
    return in_maps


TRACE_DIR = "/tmp/bass_trace"


def run(inputs, trace=False, tmpdir=None):
    """Run on hardware across 8 cores; returns (full_output, BassKernelResults)."""
    from concourse.bass_utils import run_bass_kernel_spmd

    n = np.asarray(inputs[IDX_NAMES[0]]).shape[0]
    n_core = n // N_CORES
    nc = _get_nc(n_core)
    in_maps = _make_in_maps(inputs, N_CORES, n_core)
    if trace and tmpdir is not None:
        import os
        import shutil

        shutil.rmtree(tmpdir, ignore_errors=True)
        os.makedirs(tmpdir, exist_ok=True)
    res = run_bass_kernel_spmd(nc, in_maps, core_ids=list(range(N_CORES)),
                               trace=trace, tmpdir=tmpdir if trace else None)
    out = np.empty((n, EMBED), dtype=np.float32)
    for c in range(N_CORES):
        out[c * n_core : (c + 1) * n_core, :] = res.results[c]["y"].T
    return out, res


def kernel(**inputs):
    out, _ = run(inputs)
    return out


# revision 13
# speedup vs baseline: 2.4448x; 1.1933x over previous
"""Trainium2 Bass kernel for CompoundMultivariateEmbedding (v6).

Math: out[n] = concat(level_tab[l], type_tab[t], feat_tab[f], exch_tab[e],
pair_tab[p]) @ W.T + b.  Because W is applied to a concat of block lookups,
out[n] = sum_b Ptab_b[idx_b[n]] + b where Ptab_b = tab_b @ W[:, block_b].T.

Vocab layout (K=75): level one-hot rows 0-49, pair one-hot rows 50-69,
exchange one-hot rows 70-72, plus two DIRECT rows: the 2-row type/feature
tables are affine in their index (P[t] = P[0] + t*delta), so row 73 carries
raw type_idx against delta_type and row 74 raw feat_idx against delta_feat.
The constant terms (bias + P_type[0] + P_feat[0]) are folded into the three
exchange rows (every token hits exactly one).  out.T = P.T @ st on the PE
with P [75, 128] fp16 stationary.

The host uploads idx16 [29, n_core] fp16: level idx x16, pair x8, exch x3
seed copies plus type/feat (a pure input-layout transform).  Per 8192-token
half-batch the device issues 4 strided loads + 5 depth-1 doubling DMAs to
fan the seeds out to all one-hot rows, then ONE DVE tensor_scalar is_equal
(4x mode: fp16 SBUF step-1) builds the multi-hot st [75, 8192].

Per 512 tokens one matmul accumulates P.T @ st into PSUM [128, 512]; the
PSUM fp32 -> SBUF fp16 evacuation is split DVE/ACT (1:2); HWDGE stores
1 MiB chunks to transposed y [128, n_core] fp16; the host untransposes.
"""

import sys

sys.path.insert(0, "/opt/trn_rl_repo")

import numpy as np

import concourse.bass as bass
import concourse.tile as tile
from concourse import bacc, mybir
from concourse._compat import with_exitstack
from contextlib import ExitStack

F32 = mybir.dt.float32
F16 = mybir.dt.float16

N_FULL = 1048576
N_CORES = 8
EMBED = 128

TAB_NAMES = ["level_tab", "type_tab", "feature_tab", "exchange_tab", "pair_tab"]
IDX_NAMES = ["level_idx", "type_idx", "feature_idx", "exchange_idx", "pair_idx"]
TAB_ROWS = [50, 2, 2, 3, 20]
TAB_ATTR = [25, 25, 25, 25, 28]
FOFF = [0, 25, 50, 75, 100]  # feature (W column) offset per block
# one-hot vocab offsets for level / exchange / pair
LVL0, PAIR0, EXCH0 = 0, 50, 70
DT_ROW, DF_ROW = 73, 74  # direct rows: raw type_idx, feat_idx
V = 75

# host-side idx16 seed layout: level x16, pair x8, exch x3, type, feat
SEED_LVL, SEED_PAIR, SEED_EXCH = 16, 8, 3
IDX16_ROWS = SEED_LVL + SEED_PAIR + SEED_EXCH + 2  # 29

ST = 1024  # tokens per supertile (one pso tile, 2 PSUM banks)
OSB = 4096  # tokens per output store (1 MiB fp16)
HB = 8192  # tokens per half-batch (one big is_equal)
DVE_EVAC = 3  # every DVE_EVAC-th supertile evacuates on DVE instead of ACT


@with_exitstack
def _emb_kernel(ctx, tc, y_ap, tabs, w_ap, b_ap, idx16, n_core):
    nc = tc.nc

    const = ctx.enter_context(tc.tile_pool(name="const", bufs=1))

    # ---- helpers for PE transposes ----
    pidx = const.tile([128, 1], mybir.dt.int32)
    nc.gpsimd.iota(pidx, pattern=[[0, 1]], base=0, channel_multiplier=1)
    pidx_f = const.tile([128, 1], F32)
    nc.vector.tensor_copy(pidx_f, pidx)
    iotaf = const.tile([128, 128], mybir.dt.int32)
    nc.gpsimd.iota(iotaf, pattern=[[1, 128]], base=0, channel_multiplier=0)
    ident = const.tile([128, 128], F32)
    nc.vector.tensor_scalar(ident, iotaf, pidx_f[:, :], None, mybir.AluOpType.is_equal)

    # ---- iota column: within-block index per one-hot vocab partition ----
    off_row = const.tile([1, 128], F32)
    nc.vector.memset(off_row, float("nan"))
    nc.vector.memset(off_row[:, LVL0 : LVL0 + 50], float(LVL0))
    nc.vector.memset(off_row[:, PAIR0 : PAIR0 + 20], float(PAIR0))
    nc.vector.memset(off_row[:, EXCH0 : EXCH0 + 3], float(EXCH0))
    offc = const.tile([128, 1], F32)
    nc.gpsimd.dma_start(offc, off_row)
    iota_f32 = const.tile([128, 1], F32)
    nc.vector.tensor_sub(iota_f32, pidx_f, offc)

    rep_pool = ctx.enter_context(tc.tile_pool(name="repp", bufs=3))
    st_pool = ctx.enter_context(tc.tile_pool(name="stp", bufs=3))
    out_pool = ctx.enter_context(tc.tile_pool(name="outp", bufs=2))
    n_sup = n_core // ST
    sup_per_hb = HB // ST
    sup_per_osb = OSB // ST
    n_half = n_core // HB

    # seed fan-out: (dst, src, nrows) range copies within the rep tile;
    # every row within a block is an identical copy so any range works.
    LADDER = [
        (16, 0, 16), (32, 0, 16), (48, 0, 2),   # level 16 seeds -> rows 0-49
        (58, 50, 8), (66, 50, 4),               # pair 8 seeds -> rows 50-69
    ]

    def prep_half(h):
        off = h * HB
        rep = rep_pool.tile([73, HB], F16)
        st = st_pool.tile([V, HB], F16)
        nc.gpsimd.dma_start(rep[0:16, :], idx16[0:16, off : off + HB])
        nc.gpsimd.dma_start(rep[50:58, :], idx16[16:24, off : off + HB])
        nc.gpsimd.dma_start(rep[70:73, :], idx16[24:27, off : off + HB])
        nc.gpsimd.dma_start(st[DT_ROW : DT_ROW + 2, :], idx16[27:29, off : off + HB])
        for dst, src, n in LADDER:
            nc.gpsimd.dma_start(rep[dst : dst + n, :], rep[src : src + n, :])
        nc.vector.tensor_scalar(
            st[0:73, :], rep, iota_f32[0:73, :], None, mybir.AluOpType.is_equal
        )
        return st

    st_tiles = {0: prep_half(0)}
    if n_half > 1:
        st_tiles[1] = prep_half(1)

    # ---- setup: projected tables (PSUM pool closed before the main loop) ----
    setup = ExitStack()
    psum_set = setup.enter_context(
        tc.tile_pool(name="psum_set", bufs=1, space=bass.MemorySpace.PSUM)
    )

    # W^T
    w_sb = const.tile([128, 128], F32)
    nc.sync.dma_start(w_sb, w_ap)
    psum_wt = psum_set.tile([128, 128], F32, tag="pset")
    nc.tensor.transpose(psum_wt, w_sb, ident)
    wt_sb = const.tile([128, 128], F32)
    nc.scalar.copy(wt_sb, psum_wt)

    # projected tables -> pf32 [75, 128]
    pf32 = const.tile([V, EMBED], F32)
    nc.vector.memset(pf32, 0.0)
    first_rows = {}  # j -> [1,128] tile holding P_j[0] (for type/feat fold)
    onehot_off = {0: LVL0, 3: EXCH0, 4: PAIR0}
    for j in range(5):
        rows, attr = TAB_ROWS[j], TAB_ATTR[j]
        tab_sb = const.tile([rows, attr], F32, name=f"tab{j}")
        nc.sync.dma_start(tab_sb, tabs[j])
        psum_tt = psum_set.tile([attr, rows], F32, tag="pset", name=f"ptt{j}")
        nc.tensor.transpose(psum_tt, tab_sb, ident[0:rows, 0:rows])
        tabt_sb = const.tile([attr, rows], F32, name=f"tabt{j}")
        nc.scalar.copy(tabt_sb, psum_tt)
        wb_sb = const.tile([attr, EMBED], F32, name=f"wb{j}")
        nc.gpsimd.dma_start(wb_sb, wt_sb[FOFF[j] : FOFF[j] + attr, :])
        psum_pb = psum_set.tile([rows, EMBED], F32, tag="pset", name=f"ppb{j}")
        nc.tensor.matmul(psum_pb, tabt_sb, wb_sb)
        pb_sb = const.tile([rows, EMBED], F32, name=f"pb{j}")
        nc.scalar.copy(pb_sb, psum_pb)
        if j in (1, 2):  # type / feature: direct affine rows
            rA = const.tile([1, EMBED], F32, name=f"rA{j}")
            nc.gpsimd.dma_start(rA, pb_sb[0:1, :])
            rB = const.tile([1, EMBED], F32, name=f"rB{j}")
            nc.gpsimd.dma_start(rB, pb_sb[1:2, :])
            delta = const.tile([1, EMBED], F32, name=f"dl{j}")
            nc.vector.tensor_sub(delta, rB, rA)
            row = DT_ROW if j == 1 else DF_ROW
            nc.gpsimd.dma_start(pf32[row : row + 1, :], delta)
            first_rows[j] = rA
        elif j == 3:
            # fold bias + P_type[0] + P_feat[0] into the three exchange
            # rows while they still sit at partitions 0-2 (32-aligned)
            b_row = const.tile([1, EMBED], F32)
            nc.sync.dma_start(b_row, b_ap)
            comb = const.tile([1, EMBED], F32)
            nc.vector.tensor_add(comb, first_rows[1], first_rows[2])
            nc.vector.tensor_add(comb, comb, b_row)
            comb3 = const.tile([3, EMBED], F32)
            for r in range(3):
                nc.gpsimd.dma_start(comb3[r : r + 1, :], comb)
            nc.vector.tensor_add(pb_sb, pb_sb, comb3)
            nc.gpsimd.dma_start(pf32[EXCH0 : EXCH0 + 3, :], pb_sb)
        else:
            off = onehot_off[j]
            nc.gpsimd.dma_start(pf32[off : off + rows, :], pb_sb)

    pf16 = const.tile([V, EMBED], F16)
    nc.vector.tensor_copy(pf16, pf32)

    setup.close()  # free setup PSUM banks

    # ---- main loop ----
    pso_pool = ctx.enter_context(
        tc.tile_pool(name="pout", bufs=4, space=bass.MemorySpace.PSUM)
    )
    osb = None
    for k in range(n_sup):
        h, s = divmod(k, sup_per_hb)
        if s == 0 and h + 2 < n_half:
            st_tiles[h + 2] = prep_half(h + 2)
        st_cur = st_tiles[h]
        pso = pso_pool.tile([128, ST], F32)
        for c in range(ST // 512):
            nc.tensor.matmul(
                pso[:, bass.ts(c, 512)],
                pf16,
                st_cur[:, s * ST + c * 512 : s * ST + (c + 1) * 512],
            )
        if k % sup_per_osb == 0:
            osb = out_pool.tile([128, OSB], F16)
        dst = osb[:, (k % sup_per_osb) * ST : (k % sup_per_osb + 1) * ST]
        if k % DVE_EVAC == DVE_EVAC - 1:
            nc.vector.tensor_copy(dst, pso)
        else:
            nc.scalar.copy(dst, pso)
        if (k + 1) % sup_per_osb == 0:
            n0 = (k + 1 - sup_per_osb) * ST
            nc.sync.dma_start(y_ap[:, n0 : n0 + OSB], osb)
        if s == sup_per_hb - 1:
            st_tiles.pop(h, None)


def build(n_core, num_devices=N_CORES):
    nc = bacc.Bacc(
        "TRN2", target_bir_lowering=False, debug=False, num_devices=num_devices
    )
    tabs = []
    for j, nm in enumerate(TAB_NAMES):
        tabs.append(nc.dram_tensor(nm, [TAB_ROWS[j], TAB_ATTR[j]], F32,
                                   kind="ExternalInput").ap())
    w_ap = nc.dram_tensor("W", [EMBED, EMBED], F32, kind="ExternalInput").ap()
    b_ap = nc.dram_tensor("b", [EMBED], F32, kind="ExternalInput").ap()
    idx16 = nc.dram_tensor("idx16", [IDX16_ROWS, n_core], F16,
                           kind="ExternalInput").ap()
    # transposed output: [embed, tokens] fp16; host transposes + upcasts
    y = nc.dram_tensor("y", [EMBED, n_core], F16, kind="ExternalOutput")

    with tile.TileContext(nc) as tc:
        _emb_kernel(tc, y.ap(), tabs, w_ap, b_ap, idx16, n_core)
    nc.compile()
    return nc


_NC_CACHE = {}


def _get_nc(n_core):
    if n_core not in _NC_CACHE:
        _NC_CACHE[n_core] = build(n_core)
    return _NC_CACHE[n_core]


def _make_in_maps(inputs, n_cores, n_core):
    shared = {}
    for nm in TAB_NAMES + ["W", "b"]:
        shared[nm] = np.ascontiguousarray(np.asarray(inputs[nm], dtype=np.float32))
    idx = {nm: np.asarray(inputs[nm], dtype=np.int32) for nm in IDX_NAMES}
    in_maps = []
    for c in range(n_cores):
        m = dict(shared)
        sl = slice(c * n_core, (c + 1) * n_core)
        a = np.empty((IDX16_ROWS, n_core), np.float16)
        a[0:16] = idx["level_idx"][sl].astype(np.float16)[None, :]
        a[16:24] = idx["pair_idx"][sl].astype(np.float16)[None, :]
        a[24:27] = idx["exchange_idx"][sl].astype(np.float16)[None, :]
        a[27] = idx["type_idx"][sl].astype(np.float16)
        a[28] = idx["feature_idx"][sl].astype(np.float16)
        m["idx16"] = a
        in_maps.append(m)
    return in_maps


TRACE_DIR = "/tmp/bass_trace"


def run(inputs, trace=False, tmpdir=None):
    """Run on hardware across 8 cores; returns (full_output, BassKernelResults)."""
    from concourse.bass_utils import run_bass_kernel_spmd

    n = np.asarray(inputs[IDX_NAMES[0]]).shape[0]
    n_core = n // N_CORES
    nc = _get_nc(n_core)
    in_maps = _make_in_maps(inputs, N_CORES, n_core)
    if trace and tmpdir is not None:
        import os
        import shutil

        shutil.rmtree(tmpdir, ignore_errors=True)
        os.makedirs(tmpdir, exist_ok=True)
    res = run_bass_kernel_spmd(nc, in_maps, core_ids=list(range(N_CORES)),
                               trace=trace, tmpdir=tmpdir if trace else None)
    out = np.empty((n, EMBED), dtype=np.float32)
    for c in range(N_CORES):
        out[c * n_core : (c + 1) * n_core, :] = res.results[c]["y"].T
    return out, res


def kernel(**inputs):
    out, _ = run(inputs)
    return out


# revision 14
# speedup vs baseline: 2.4493x; 1.0018x over previous
"""Trainium2 Bass kernel for CompoundMultivariateEmbedding (v6).

Math: out[n] = concat(level_tab[l], type_tab[t], feat_tab[f], exch_tab[e],
pair_tab[p]) @ W.T + b.  Because W is applied to a concat of block lookups,
out[n] = sum_b Ptab_b[idx_b[n]] + b where Ptab_b = tab_b @ W[:, block_b].T.

Vocab layout (K=75): level one-hot rows 0-49, pair one-hot rows 50-69,
exchange one-hot rows 70-72, plus two DIRECT rows: the 2-row type/feature
tables are affine in their index (P[t] = P[0] + t*delta), so row 73 carries
raw type_idx against delta_type and row 74 raw feat_idx against delta_feat.
The constant terms (bias + P_type[0] + P_feat[0]) are folded into the three
exchange rows (every token hits exactly one).  out.T = P.T @ st on the PE
with P [75, 128] fp16 stationary.

The host uploads idx16 [29, n_core] fp16: level idx x16, pair x8, exch x3
seed copies plus type/feat (a pure input-layout transform).  Per 8192-token
half-batch the device issues 4 strided loads + 5 depth-1 doubling DMAs to
fan the seeds out to all one-hot rows, then ONE DVE tensor_scalar is_equal
(4x mode: fp16 SBUF step-1) builds the multi-hot st [75, 8192].

Per 512 tokens one matmul accumulates P.T @ st into PSUM [128, 512]; the
PSUM fp32 -> SBUF fp16 evacuation is split DVE/ACT (1:2); HWDGE stores
1 MiB chunks to transposed y [128, n_core] fp16; the host untransposes.
"""

import sys

sys.path.insert(0, "/opt/trn_rl_repo")

import numpy as np

import concourse.bass as bass
import concourse.tile as tile
from concourse import bacc, mybir
from concourse._compat import with_exitstack
from contextlib import ExitStack

F32 = mybir.dt.float32
F16 = mybir.dt.float16

N_FULL = 1048576
N_CORES = 8
EMBED = 128

TAB_NAMES = ["level_tab", "type_tab", "feature_tab", "exchange_tab", "pair_tab"]
IDX_NAMES = ["level_idx", "type_idx", "feature_idx", "exchange_idx", "pair_idx"]
TAB_ROWS = [50, 2, 2, 3, 20]
TAB_ATTR = [25, 25, 25, 25, 28]
FOFF = [0, 25, 50, 75, 100]  # feature (W column) offset per block
# one-hot vocab offsets for level / exchange / pair
LVL0, PAIR0, EXCH0 = 0, 50, 70
DT_ROW, DF_ROW = 73, 74  # direct rows: raw type_idx, feat_idx
V = 75

# host-side idx16 seed layout: level x16, pair x8, exch x3, type, feat
SEED_LVL, SEED_PAIR, SEED_EXCH = 16, 8, 3
IDX16_ROWS = SEED_LVL + SEED_PAIR + SEED_EXCH + 2  # 29

ST = 1024  # tokens per supertile (one pso tile, 2 PSUM banks)
OSB = 4096  # tokens per output store (1 MiB fp16)
HB = 8192  # tokens per half-batch (one big is_equal)
DVE_EVAC = 3  # every DVE_EVAC-th supertile evacuates on DVE instead of ACT


@with_exitstack
def _emb_kernel(ctx, tc, y_ap, tabs, w_ap, b_ap, idx16, n_core):
    nc = tc.nc

    const = ctx.enter_context(tc.tile_pool(name="const", bufs=1))

    # ---- helpers for PE transposes ----
    pidx = const.tile([128, 1], mybir.dt.int32)
    nc.gpsimd.iota(pidx, pattern=[[0, 1]], base=0, channel_multiplier=1)
    pidx_f = const.tile([128, 1], F32)
    nc.vector.tensor_copy(pidx_f, pidx)
    iotaf = const.tile([128, 128], mybir.dt.int32)
    nc.gpsimd.iota(iotaf, pattern=[[1, 128]], base=0, channel_multiplier=0)
    ident = const.tile([128, 128], F32)
    nc.vector.tensor_scalar(ident, iotaf, pidx_f[:, :], None, mybir.AluOpType.is_equal)

    # ---- iota column: within-block index per one-hot vocab partition ----
    off_row = const.tile([1, 128], F32)
    nc.vector.memset(off_row, float("nan"))
    nc.vector.memset(off_row[:, LVL0 : LVL0 + 50], float(LVL0))
    nc.vector.memset(off_row[:, PAIR0 : PAIR0 + 20], float(PAIR0))
    nc.vector.memset(off_row[:, EXCH0 : EXCH0 + 3], float(EXCH0))
    offc = const.tile([128, 1], F32)
    nc.gpsimd.dma_start(offc, off_row)
    iota_f32 = const.tile([128, 1], F32)
    nc.vector.tensor_sub(iota_f32, pidx_f, offc)

    rep_pool = ctx.enter_context(tc.tile_pool(name="repp", bufs=3))
    st_pool = ctx.enter_context(tc.tile_pool(name="stp", bufs=3))
    out_pool = ctx.enter_context(tc.tile_pool(name="outp", bufs=3))
    n_sup = n_core // ST
    sup_per_hb = HB // ST
    sup_per_osb = OSB // ST
    n_half = n_core // HB

    # seed fan-out: (dst, src, nrows) range copies within the rep tile;
    # every row within a block is an identical copy so any range works.
    LADDER = [
        (16, 0, 16), (32, 0, 16), (48, 0, 2),   # level 16 seeds -> rows 0-49
        (58, 50, 8), (66, 50, 4),               # pair 8 seeds -> rows 50-69
    ]

    def prep_dma(h):
        off = h * HB
        rep = rep_pool.tile([73, HB], F16)
        st = st_pool.tile([V, HB], F16)
        nc.gpsimd.dma_start(rep[0:16, :], idx16[0:16, off : off + HB])
        nc.gpsimd.dma_start(rep[50:58, :], idx16[16:24, off : off + HB])
        nc.sync.dma_start(rep[70:73, :], idx16[24:27, off : off + HB])
        nc.sync.dma_start(st[DT_ROW : DT_ROW + 2, :], idx16[27:29, off : off + HB])
        for dst, src, n in LADDER:
            nc.gpsimd.dma_start(rep[dst : dst + n, :], rep[src : src + n, :])
        return rep, st

    def prep_iseq(h):
        rep, st = half_tiles[h]
        nc.vector.tensor_scalar(
            st[0:73, :], rep, iota_f32[0:73, :], None, mybir.AluOpType.is_equal
        )

    half_tiles = {0: prep_dma(0)}
    if n_half > 1:
        half_tiles[1] = prep_dma(1)
    prep_iseq(0)

    # ---- setup: projected tables (PSUM pool closed before the main loop) ----
    setup = ExitStack()
    psum_set = setup.enter_context(
        tc.tile_pool(name="psum_set", bufs=1, space=bass.MemorySpace.PSUM)
    )

    # W^T
    w_sb = const.tile([128, 128], F32)
    nc.sync.dma_start(w_sb, w_ap)
    psum_wt = psum_set.tile([128, 128], F32, tag="pset")
    nc.tensor.transpose(psum_wt, w_sb, ident)
    wt_sb = const.tile([128, 128], F32)
    nc.scalar.copy(wt_sb, psum_wt)

    # projected tables -> pf32 [75, 128]
    pf32 = const.tile([V, EMBED], F32)
    nc.vector.memset(pf32, 0.0)
    first_rows = {}  # j -> [1,128] tile holding P_j[0] (for type/feat fold)
    onehot_off = {0: LVL0, 3: EXCH0, 4: PAIR0}
    for j in range(5):
        rows, attr = TAB_ROWS[j], TAB_ATTR[j]
        tab_sb = const.tile([rows, attr], F32, name=f"tab{j}")
        nc.sync.dma_start(tab_sb, tabs[j])
        psum_tt = psum_set.tile([attr, rows], F32, tag="pset", name=f"ptt{j}")
        nc.tensor.transpose(psum_tt, tab_sb, ident[0:rows, 0:rows])
        tabt_sb = const.tile([attr, rows], F32, name=f"tabt{j}")
        nc.scalar.copy(tabt_sb, psum_tt)
        wb_sb = const.tile([attr, EMBED], F32, name=f"wb{j}")
        nc.gpsimd.dma_start(wb_sb, wt_sb[FOFF[j] : FOFF[j] + attr, :])
        psum_pb = psum_set.tile([rows, EMBED], F32, tag="pset", name=f"ppb{j}")
        nc.tensor.matmul(psum_pb, tabt_sb, wb_sb)
        pb_sb = const.tile([rows, EMBED], F32, name=f"pb{j}")
        nc.scalar.copy(pb_sb, psum_pb)
        if j in (1, 2):  # type / feature: direct affine rows
            rA = const.tile([1, EMBED], F32, name=f"rA{j}")
            nc.gpsimd.dma_start(rA, pb_sb[0:1, :])
            rB = const.tile([1, EMBED], F32, name=f"rB{j}")
            nc.gpsimd.dma_start(rB, pb_sb[1:2, :])
            delta = const.tile([1, EMBED], F32, name=f"dl{j}")
            nc.vector.tensor_sub(delta, rB, rA)
            row = DT_ROW if j == 1 else DF_ROW
            nc.gpsimd.dma_start(pf32[row : row + 1, :], delta)
            first_rows[j] = rA
        elif j == 3:
            # fold bias + P_type[0] + P_feat[0] into the three exchange
            # rows while they still sit at partitions 0-2 (32-aligned)
            b_row = const.tile([1, EMBED], F32)
            nc.sync.dma_start(b_row, b_ap)
            comb = const.tile([1, EMBED], F32)
            nc.vector.tensor_add(comb, first_rows[1], first_rows[2])
            nc.vector.tensor_add(comb, comb, b_row)
            comb3 = const.tile([3, EMBED], F32)
            for r in range(3):
                nc.gpsimd.dma_start(comb3[r : r + 1, :], comb)
            nc.vector.tensor_add(pb_sb, pb_sb, comb3)
            nc.gpsimd.dma_start(pf32[EXCH0 : EXCH0 + 3, :], pb_sb)
        else:
            off = onehot_off[j]
            nc.gpsimd.dma_start(pf32[off : off + rows, :], pb_sb)

    pf16 = const.tile([V, EMBED], F16)
    nc.vector.tensor_copy(pf16, pf32)

    setup.close()  # free setup PSUM banks

    # ---- main loop ----
    pso_pool = ctx.enter_context(
        tc.tile_pool(name="pout", bufs=4, space=bass.MemorySpace.PSUM)
    )
    osb = None
    for k in range(n_sup):
        h, s = divmod(k, sup_per_hb)
        if s == 0 and h + 2 < n_half:
            half_tiles[h + 2] = prep_dma(h + 2)
        if s == 5 and h + 1 < n_half:
            prep_iseq(h + 1)
        st_cur = half_tiles[h][1]
        pso = pso_pool.tile([128, ST], F32)
        for c in range(ST // 512):
            nc.tensor.matmul(
                pso[:, bass.ts(c, 512)],
                pf16,
                st_cur[:, s * ST + c * 512 : s * ST + (c + 1) * 512],
            )
        if k % sup_per_osb == 0:
            osb = out_pool.tile([128, OSB], F16)
        dst = osb[:, (k % sup_per_osb) * ST : (k % sup_per_osb + 1) * ST]
        if k % DVE_EVAC == DVE_EVAC - 1:
            nc.vector.tensor_copy(dst, pso)
        else:
            nc.scalar.copy(dst, pso)
        if (k + 1) % sup_per_osb == 0:
            n0 = (k + 1 - sup_per_osb) * ST
            nc.sync.dma_start(y_ap[:, n0 : n0 + OSB], osb)
        if s == sup_per_hb - 1:
            half_tiles.pop(h, None)


def build(n_core, num_devices=N_CORES):
    nc = bacc.Bacc(
        "TRN2", target_bir_lowering=False, debug=False, num_devices=num_devices
    )
    tabs = []
    for j, nm in enumerate(TAB_NAMES):
        tabs.append(nc.dram_tensor(nm, [TAB_ROWS[j], TAB_ATTR[j]], F32,
                                   kind="ExternalInput").ap())
    w_ap = nc.dram_tensor("W", [EMBED, EMBED], F32, kind="ExternalInput").ap()
    b_ap = nc.dram_tensor("b", [EMBED], F32, kind="ExternalInput").ap()
    idx16 = nc.dram_tensor("idx16", [IDX16_ROWS, n_core], F16,
                           kind="ExternalInput").ap()
    # transposed output: [embed, tokens] fp16; host transposes + upcasts
    y = nc.dram_tensor("y", [EMBED, n_core], F16, kind="ExternalOutput")

    with tile.TileContext(nc) as tc:
        _emb_kernel(tc, y.ap(), tabs, w_ap, b_ap, idx16, n_core)
    nc.compile()
    return nc


_NC_CACHE = {}


def _get_nc(n_core):
    if n_core not in _NC_CACHE:
        _NC_CACHE[n_core] = build(n_core)
    return _NC_CACHE[n_core]


def _make_in_maps(inputs, n_cores, n_core):
    shared = {}
    for nm in TAB_NAMES + ["W", "b"]:
        shared[nm] = np.ascontiguousarray(np.asarray(inputs[nm], dtype=np.float32))
    idx = {nm: np.asarray(inputs[nm], dtype=np.int32) for nm in IDX_NAMES}
    in_maps = []
    for c in range(n_cores):
        m = dict(shared)
        sl = slice(c * n_core, (c + 1) * n_core)
        a = np.empty((IDX16_ROWS, n_core), np.float16)
        a[0:16] = idx["level_idx"][sl].astype(np.float16)[None, :]
        a[16:24] = idx["pair_idx"][sl].astype(np.float16)[None, :]
        a[24:27] = idx["exchange_idx"][sl].astype(np.float16)[None, :]
        a[27] = idx["type_idx"][sl].astype(np.float16)
        a[28] = idx["feature_idx"][sl].astype(np.float16)
        m["idx16"] = a
        in_maps.append(m)
    return in_maps


TRACE_DIR = "/tmp/bass_trace"


def run(inputs, trace=False, tmpdir=None):
    """Run on hardware across 8 cores; returns (full_output, BassKernelResults)."""
    from concourse.bass_utils import run_bass_kernel_spmd

    n = np.asarray(inputs[IDX_NAMES[0]]).shape[0]
    n_core = n // N_CORES
    nc = _get_nc(n_core)
    in_maps = _make_in_maps(inputs, N_CORES, n_core)
    if trace and tmpdir is not None:
        import os
        import shutil

        shutil.rmtree(tmpdir, ignore_errors=True)
        os.makedirs(tmpdir, exist_ok=True)
    res = run_bass_kernel_spmd(nc, in_maps, core_ids=list(range(N_CORES)),
                               trace=trace, tmpdir=tmpdir if trace else None)
    out = np.empty((n, EMBED), dtype=np.float32)
    for c in range(N_CORES):
        out[c * n_core : (c + 1) * n_core, :] = res.results[c]["y"].T
    return out, res


def kernel(**inputs):
    out, _ = run(inputs)
    return out
